# revision 1
# baseline (speedup 1.0000x reference)
"""ChannelAttentionPropagation1D kernel for 8x TRN2 NeuronCores.

Reference computation (per batch b):
  kv[c,d]   = sum_{t,n} key_mem[b,t,n,c] * val_mem[b,t,n,d]    # (64, 64)
  kv_soft   = softmax(kv, axis=c)
  out[n,d]  = alpha * (key_cur[b] @ kv_soft)[n,d] + val_cur[b,n,d]

Sharding (8 cores):
  phase 1: core i contracts the t=i slice of key_mem/val_mem (16384 tokens
           per batch) into a partial kv^T, then AllReduce (64 KB) over cores.
  phase 2: core i computes the n-slice [2048*i, 2048*(i+1)) of the output.

Layout notes:
  - phase 1 accumulates kvT[d,c] (PSUM) so the softmax axis c lands on the
    free axis; a tiny PE transpose afterwards yields kv_soft[c,d].
  - key_cur is transposed (and scaled by alpha) on the host so its channel
    axis is the SBUF partition axis; its token axis is permuted n = 16p + j
    so phase-2 output tiles assemble into 4KB-contiguous-per-partition
    stores.
"""

import numpy as np

import concourse.bacc as bacc
import concourse.mybir as mybir
import concourse.tile as tile
from concourse import bass_utils, masks

F32 = mybir.dt.float32

N_CORES = 8
N, T, NTOK, C, C2 = 4, 8, 16384, 64, 64
NSL = NTOK // N_CORES  # 2048: phase-2 token slice per core
A_TILES = 64           # 128-token matmul tiles per half-batch chunk
HALF = NTOK // 2       # 8192 tokens per phase-1 DMA chunk

_CACHE = {}

# Extra kwargs forwarded to run_bass_kernel_spmd (used by the profiling
# harness to request an NTFF trace; empty for normal correctness runs).
_RUN_OPTS = {}


def _build_program():
    nc = bacc.Bacc(
        "TRN2",
        target_bir_lowering=False,
        debug=False,
        enable_asserts=False,
        num_devices=N_CORES,
    )

    km = nc.dram_tensor("key_mem", [N, NTOK, C], F32, kind="ExternalInput").ap()
    vm = nc.dram_tensor("val_mem", [N, NTOK, C2], F32, kind="ExternalInput").ap()
    # key_curT is host-packed [128, NSL/2]: rows 0:64 = channels for output
    # tiles j=0..7, rows 64:128 = channels for tiles j=8..15 (row-tiled
    # phase-2 pairs).
    kct = nc.dram_tensor(
        "key_curT", [N, 128, NSL // 2], F32, kind="ExternalInput"
    ).ap()
    vc = nc.dram_tensor("val_cur", [N, NSL, C2], F32, kind="ExternalInput").ap()
    out = nc.dram_tensor("out", [N, NSL, C2], F32, kind="ExternalOutput").ap()

    with tile.TileContext(nc) as tc:
        with (
            tc.tile_pool(name="persist", bufs=1) as persist,
            tc.tile_pool(name="big", bufs=4) as big,
            tc.tile_pool(name="tmp", bufs=2) as tmp,
            tc.tile_pool(name="stage", bufs=2) as stage_pool,
            tc.tile_pool(name="ps", bufs=2, space="PSUM") as ps,
            tc.tile_pool(name="dram", bufs=1, space="DRAM") as dram,
        ):
            ident = persist.tile([128, 128], F32)
            masks.make_identity(nc, ident[:])

            kct_sb = persist.tile([128, N * (NSL // 2)], F32)
            vc_sb = persist.tile([128, N * (NSL // 128) * C2], F32)

            kvt_sb = persist.tile([C2, N * C], F32)
            kvt_all = persist.tile([C2, N * N_CORES * C], F32)
            kvt_red = persist.tile([C2, N * C], F32)
            kv_soft = persist.tile([128, N * C2], F32)
            ar_outs = {}

            def emit_tails():
                """AR readbacks + softmax + transpose + phase 2 + stores for
                all batches, emitted STAGE-MAJOR: engine FIFOs run in program
                order, so batch-major emission would serialize the four
                ~15us-latency chains. Stage-major lets the four batches
                pipeline through gpsimd/DVE/ACT/PE. All tails sit after the
                whole phase-1 so a late AllReduce (peer-core launch skew can
                exceed 100us) never blocks local phase-1 work."""
                # readbacks ride the sync queue: its chunk DMAs have drained
                # by now, while gpsimd still holds doorbell-3 (which waits
                # for the end of phase-1) and scalar holds ar_in3. Each
                # AllGather result is [rank, d, c]; pull it into SBUF as
                # [d, (rank c)] and tree-reduce with 3 DVE adds per batch.
                W = N_CORES * C
                for b in range(N):
                    nc.sync.dma_start(
                        kvt_all[:, b * W:(b + 1) * W].rearrange(
                            "d (r c) -> d r c", r=N_CORES
                        ),
                        ar_outs[b].rearrange("r d c -> d r c"),
                    )
                for width in (4 * C, 2 * C):
                    for b in range(N):
                        lo = kvt_all[:, b * W: b * W + width]
                        nc.vector.tensor_add(
                            lo, lo, kvt_all[:, b * W + width: b * W + 2 * width]
                        )
                for b in range(N):
                    nc.vector.tensor_add(
                        kvt_red[:, b * C:(b + 1) * C],
                        kvt_all[:, b * W: b * W + C],
                        kvt_all[:, b * W + C: b * W + 2 * C],
                    )
                neg_mx = tmp.tile([C2, N], F32)
                for b in range(N):
                    nc.vector.reduce_max(
                        out=neg_mx[:, b:b + 1],
                        in_=kvt_red[:, b * C:(b + 1) * C],
                        axis=mybir.AxisListType.X,
                        negate=True,
                    )
                ex = tmp.tile([C2, N * C], F32)
                sm = tmp.tile([C2, N], F32)
                for b in range(N):
                    nc.scalar.activation(
                        ex[:, b * C:(b + 1) * C],
                        kvt_red[:, b * C:(b + 1) * C],
                        mybir.ActivationFunctionType.Exp,
                        bias=neg_mx[:, b:b + 1], scale=1.0,
                        accum_out=sm[:, b:b + 1],
                    )
                rv = tmp.tile([C2, N], F32)
                for b in range(N):
                    nc.vector.reciprocal(rv[:, b:b + 1], sm[:, b:b + 1])
                for b in range(N):
                    nc.vector.tensor_scalar_mul(
                        ex[:, b * C:(b + 1) * C],
                        ex[:, b * C:(b + 1) * C],
                        rv[:, b:b + 1],
                    )
                # Transpose softmaxed kvT to kv[c, d] (transpose-mode matmul
                # must write PSUM partition 0), then mirror the whole strip
                # into partitions 64:128 with one SBUF->SBUF DMA so row-tiled
                # phase-2 can read kv from the upper rows too.
                for b in range(N):
                    tp = ps.tile([C, C2], F32, tag="tp", name=f"tp{b}", bufs=2)
                    nc.tensor.transpose(
                        tp[:], ex[:, b * C:(b + 1) * C], ident[0:C2, 0:C2]
                    )
                    nc.vector.tensor_copy(
                        kv_soft[0:C, b * C2:(b + 1) * C2], tp[:]
                    )
                nc.sync.dma_start(kv_soft[64:64 + C, :], kv_soft[0:C, :])
                stgs = {}
                for b in range(N):
                    stgs[b] = stage_pool.tile(
                        [128, (NSL // 128) * C2], F32, tag=f"stg{b}",
                        name=f"stg{b}",
                    )
                # Row-tiled phase 2: tile j contracts on PE rows 0:64
                # (kct rows 0:64, kv rows 0:64), tile j+8 on rows 64:128 —
                # the two matmuls run concurrently on separate subarrays.
                HNSL = NSL // 2
                for b in range(N):
                    for j in range(8):
                        col = slice(b * HNSL + j * 128, b * HNSL + (j + 1) * 128)
                        o_a = ps.tile(
                            [128, C2], F32, tag="o", name=f"oa{b}_{j}", bufs=4
                        )
                        nc.tensor.matmul(
                            o_a[:],
                            lhsT=kct_sb[0:C, col],
                            rhs=kv_soft[0:C, b * C2:(b + 1) * C2],
                            start=True,
                            stop=True,
                            tile_position=(0, 0),
                        )
                        o_b = ps.tile(
                            [128, C2], F32, tag="o", name=f"ob{b}_{j}", bufs=4
                        )
                        nc.tensor.matmul(
                            o_b[:],
                            lhsT=kct_sb[64:64 + C, col],
                            rhs=kv_soft[64:64 + C, b * C2:(b + 1) * C2],
                            start=True,
                            stop=True,
                            tile_position=(64, 0),
                        )
                        nc.vector.tensor_add(
                            stgs[b][:, j * C2:(j + 1) * C2],
                            o_a[:],
                            vc_sb[:, b * 1024 + j * C2: b * 1024 + (j + 1) * C2],
                        )
                        nc.vector.tensor_add(
                            stgs[b][:, (j + 8) * C2:(j + 9) * C2],
                            o_b[:],
                            vc_sb[:, b * 1024 + (j + 8) * C2: b * 1024 + (j + 9) * C2],
                        )
                    # split the store so the second half overlaps the
                    # remaining adds (trims the last batch's tail)
                    oap = out[b].rearrange("(p j) c -> p (j c)", p=128)
                    nc.sync.dma_start(oap[:, 0:8 * C2], stgs[b][:, 0:8 * C2])
                    nc.sync.dma_start(
                        oap[:, 8 * C2:16 * C2], stgs[b][:, 8 * C2:16 * C2]
                    )

            # ---- phase 1: partial kvT[d, c] per batch, col-tiled 2x ----
            # Even token-tiles accumulate on PE column group 0 (psum rows
            # 0:64), odd tiles on column group 2 (psum rows 64:128); the two
            # halves' LDWEIGHTS/MATMUL overlap on independent subarrays.
            for b in range(N):
                kv_ps = ps.tile([128, C], F32, tag="kv", name=f"kv{b}")
                for h in range(2):
                    k_sb = big.tile([128, HALF // 128 * C], F32, tag="k")
                    v_sb = big.tile([128, HALF // 128 * C2], F32, tag="v")
                    sl = slice(h * HALF, (h + 1) * HALF)
                    nc.sync.dma_start(
                        k_sb[:], km[b, sl, :].rearrange("(p a) c -> p (a c)", p=128)
                    )
                    nc.sync.dma_start(
                        v_sb[:], vm[b, sl, :].rearrange("(p a) c -> p (a c)", p=128)
                    )
                    if h == 1:
                        # phase-2 inputs for batch b: issued on the scalar
                        # (ACT) DMA FIFO so they never delay the phase-1
                        # chunk stream on the sync FIFO.
                        nc.scalar.dma_start(
                            kct_sb[:, b * (NSL // 2):(b + 1) * (NSL // 2)],
                            kct[b],
                        )
                        nc.scalar.dma_start(
                            vc_sb[:, b * 1024:(b + 1) * 1024],
                            vc[b].rearrange("(p j) c -> p (j c)", p=128),
                        )
                    for a in range(A_TILES):
                        half = a % 2
                        nc.tensor.matmul(
                            kv_ps[64 * half:64 * half + C2, :],
                            lhsT=v_sb[:, a * C2:(a + 1) * C2],
                            rhs=k_sb[:, a * C:(a + 1) * C],
                            start=(h == 0 and a < 2),
                            stop=(h == 1 and a >= A_TILES - 2),
                            tile_position=(0, 64 * half),
                        )
                # partial kvT = even-half + odd-half (DVE can read only one
                # PSUM operand per instruction, so copy then add)
                nc.vector.tensor_copy(kvt_sb[:, b * C:(b + 1) * C], kv_ps[0:C2, :])
                nc.vector.tensor_add(
                    kvt_sb[:, b * C:(b + 1) * C],
                    kvt_sb[:, b * C:(b + 1) * C],
                    kv_ps[64:64 + C2, :],
                )
                # per-batch AllGather (cheaper than AllReduce on the CC
                # core); the 8 partials are tree-reduced locally on DVE.
                ar_in = dram.tile([C2, C], F32, tag=f"ar_in{b}", name=f"ar_in{b}")
                ar_out = dram.tile(
                    [N_CORES, C2, C], F32, addr_space="Shared", tag=f"ar_out{b}",
                    name=f"ar_out{b}",
                )
                ar_outs[b] = ar_out
                nc.scalar.dma_start(ar_in[:], kvt_sb[:, b * C:(b + 1) * C])
                nc.gpsimd.collective_compute(
                    "AllGather",
                    mybir.AluOpType.bypass,
                    replica_groups=[list(range(N_CORES))],
                    ins=[ar_in.opt()],
                    outs=[ar_out.opt()],
                )
            emit_tails()

    nc.compile()
    return nc


def _get_program():
    if "nc" not in _CACHE:
        _CACHE["nc"] = _build_program()
    return _CACHE["nc"]


def kernel(key_mem, val_mem, key_cur, val_cur, alpha):
    key_mem = np.asarray(key_mem, dtype=np.float32)
    val_mem = np.asarray(val_mem, dtype=np.float32)
    key_cur = np.asarray(key_cur, dtype=np.float32)
    val_cur = np.asarray(val_cur, dtype=np.float32)
    alpha_f = float(np.asarray(alpha).reshape(-1)[0])

    nc = _get_program()

    # key_cur^T with alpha folded in; token axis permuted so that SBUF
    # column j*128+p holds token p*16+j (phase-2 store contiguity).
    kc_scaled = (alpha_f * key_cur).astype(np.float32)
    in_maps = []
    for i in range(N_CORES):
        kct_i = kc_scaled[:, i * NSL:(i + 1) * NSL, :].transpose(0, 2, 1)
        kct_i = (
            kct_i.reshape(N, C, 128, NSL // 128)
            .transpose(0, 1, 3, 2)
            .reshape(N, C, NSL)
        )
        # pack for row-tiled phase 2: rows 0:64 = tiles j=0..7,
        # rows 64:128 = tiles j=8..15
        kct_i = (
            kct_i.reshape(N, C, 2, NSL // 2)
            .transpose(0, 2, 1, 3)
            .reshape(N, 128, NSL // 2)
        )
        in_maps.append(
            {
                "key_mem": np.ascontiguousarray(key_mem[:, i]),
                "val_mem": np.ascontiguousarray(val_mem[:, i]),
                "key_curT": np.ascontiguousarray(kct_i),
                "val_cur": np.ascontiguousarray(val_cur[:, i * NSL:(i + 1) * NSL, :]),
            }
        )

    res = bass_utils.run_bass_kernel_spmd(
        nc, in_maps, core_ids=list(range(N_CORES)), **_RUN_OPTS
    )
    _CACHE["last_result"] = res
    outs = [res.results[i]["out"] for i in range(N_CORES)]
    return np.concatenate(outs, axis=1).astype(np.float32)



# revision 3
# speedup vs baseline: 1.4588x; 1.4588x over previous
"""ChannelAttentionPropagation1D kernel for 8x TRN2 NeuronCores.

Reference computation (per batch b):
  kv[c,d]   = sum_{t,n} key_mem[b,t,n,c] * val_mem[b,t,n,d]    # (64, 64)
  kv_soft   = softmax(kv, axis=c)
  out[n,d]  = alpha * (key_cur[b] @ kv_soft)[n,d] + val_cur[b,n,d]

Sharding (8 cores):
  phase 1: core i contracts the t=i slice of key_mem/val_mem (16384 tokens
           per batch) into a partial kv^T, AllGathered over cores in two
           2-batch groups.
  phase 2: core i computes the n-slice [2048*i, 2048*(i+1)) of the output.

Precision: key_mem/val_mem/key_cur are cast to fp16 on the host (empirical
rel-fro error 4e-5 on the reference data, far under the 2e-2 gate: the kv
logits have std ~600 so the softmax is near-one-hot and fp16 never flips an
argmax; the fp16 matmuls accumulate in fp32 PSUM). This halves the dominant
HBM stream and takes the PE off the 4x-slower fp32 path.

Layout notes:
  - phase 1 accumulates kvT[d,c] (PSUM) so the softmax axis c lands on the
    free axis; a tiny PE transpose afterwards yields kv_soft[c,d] (fp16).
  - all DRAM operands are host-packed to the exact SBUF layout so every DMA
    is a dense [128, rowbytes] block copy.
  - key_cur is transposed (and scaled by alpha) on the host so its channel
    axis is the SBUF partition axis; its token axis is permuted n = 16p + j
    so phase-2 output tiles assemble into 4KB-contiguous-per-partition
    stores.
"""

import numpy as np

import concourse.bacc as bacc
import concourse.mybir as mybir
import concourse.tile as tile
from concourse import bass_utils, masks

F32 = mybir.dt.float32
F16 = mybir.dt.float16

N_CORES = 8
N, T, NTOK, C, C2 = 4, 8, 16384, 64, 64
NSL = NTOK // N_CORES  # 2048: phase-2 token slice per core
A_TILES = 64           # 128-token matmul tiles per half-batch chunk
HALF = NTOK // 2       # 8192 tokens per phase-1 DMA chunk

_CACHE = {}

# Extra kwargs forwarded to run_bass_kernel_spmd (used by the profiling
# harness to request an NTFF trace; empty for normal correctness runs).
_RUN_OPTS = {}


def _build_program():
    nc = bacc.Bacc(
        "TRN2",
        target_bir_lowering=False,
        debug=False,
        enable_asserts=False,
        num_devices=N_CORES,
    )

    # host-packed [b, h, p, a, c]: token t = h*8192 + p*64 + a
    km = nc.dram_tensor(
        "key_mem", [N, 2, 128, A_TILES * C], F16, kind="ExternalInput"
    ).ap()
    vm = nc.dram_tensor(
        "val_mem", [N, 2, 128, A_TILES * C2], F16, kind="ExternalInput"
    ).ap()
    # key_curT is host-packed [128, NSL/2]: rows 0:64 = channels for output
    # tiles j=0..7, rows 64:128 = channels for tiles j=8..15 (row-tiled
    # phase-2 pairs).
    kct = nc.dram_tensor(
        "key_curT", [N, 128, NSL // 2], F16, kind="ExternalInput"
    ).ap()
    vc = nc.dram_tensor("val_cur", [N, NSL, C2], F32, kind="ExternalInput").ap()
    out = nc.dram_tensor("out", [N, NSL, C2], F32, kind="ExternalOutput").ap()

    with tile.TileContext(nc) as tc:
        with (
            tc.tile_pool(name="persist", bufs=1) as persist,
            tc.tile_pool(name="big", bufs=4) as big,
            tc.tile_pool(name="tmp", bufs=2) as tmp,
            tc.tile_pool(name="stage", bufs=2) as stage_pool,
            tc.tile_pool(name="ps", bufs=2, space="PSUM") as ps,
            tc.tile_pool(name="dram", bufs=1, space="DRAM") as dram,
        ):
            ident = persist.tile([128, 128], F32)
            masks.make_identity(nc, ident[:])

            kct_sb = persist.tile([128, N * (NSL // 2)], F16)
            vc_sb = persist.tile([128, N * (NSL // 128) * C2], F32)

            kvt_sb = persist.tile([C2, N * C], F32)
            # per 2-batch AllGather group: [d, (rank, 2*C)]
            kvt_all = persist.tile([C2, 2 * N_CORES * 2 * C], F32)
            kvt_red = persist.tile([C2, N * C], F32)
            kv_soft = persist.tile([128, N * C2], F16)
            ar_outs = {}

            def emit_tails():
                """AR readbacks + group tree-reduce + softmax + transpose +
                phase 2 + stores, emitted STAGE-MAJOR after all of phase 1:
                engine FIFOs run in program order, so this keeps a late
                AllGather (peer-core launch skew) from ever blocking local
                phase-1 work, while the four batches pipeline through
                DVE/ACT/PE."""
                GW = N_CORES * 2 * C  # 1024: group width in kvt_all
                for g in range(2):
                    nc.sync.dma_start(
                        kvt_all[:, g * GW:(g + 1) * GW].rearrange(
                            "d (r c) -> d r c", r=N_CORES
                        ),
                        ar_outs[g].rearrange("r d c -> d r c"),
                    )
                # tree-reduce the 8 ranks (each 2*C=128 cols wide): widths
                # 512/256, then final 128 into kvt_red (kvt_red cols b*C
                # match batches 2g, 2g+1)
                for width in (8 * C, 4 * C):
                    for g in range(2):
                        lo = kvt_all[:, g * GW: g * GW + 2 * width]
                        nc.vector.tensor_add(
                            lo[:, 0:width], lo[:, 0:width], lo[:, width:2 * width]
                        )
                for g in range(2):
                    nc.vector.tensor_add(
                        kvt_red[:, g * 2 * C:(g + 1) * 2 * C],
                        kvt_all[:, g * GW: g * GW + 2 * C],
                        kvt_all[:, g * GW + 2 * C: g * GW + 4 * C],
                    )
                neg_mx = tmp.tile([C2, N], F32)
                for b in range(N):
                    nc.vector.reduce_max(
                        out=neg_mx[:, b:b + 1],
                        in_=kvt_red[:, b * C:(b + 1) * C],
                        axis=mybir.AxisListType.X,
                        negate=True,
                    )
                ex = tmp.tile([C2, N * C], F32)
                sm = tmp.tile([C2, N], F32)
                for b in range(N):
                    nc.scalar.activation(
                        ex[:, b * C:(b + 1) * C],
                        kvt_red[:, b * C:(b + 1) * C],
                        mybir.ActivationFunctionType.Exp,
                        bias=neg_mx[:, b:b + 1], scale=1.0,
                        accum_out=sm[:, b:b + 1],
                    )
                rv = tmp.tile([C2, N], F32)
                for b in range(N):
                    nc.vector.reciprocal(rv[:, b:b + 1], sm[:, b:b + 1])
                for b in range(N):
                    nc.vector.tensor_scalar_mul(
                        ex[:, b * C:(b + 1) * C],
                        ex[:, b * C:(b + 1) * C],
                        rv[:, b:b + 1],
                    )
                # Transpose softmaxed kvT to kv[c, d] (transpose-mode matmul
                # must write PSUM partition 0); the PSUM->SBUF copy casts to
                # fp16 for the phase-2 matmul. Then mirror the strip into
                # partitions 64:128 with one SBUF->SBUF DMA so row-tiled
                # phase-2 can read kv from the upper rows too.
                for b in range(N):
                    tp = ps.tile([C, C2], F32, tag="tp", name=f"tp{b}", bufs=2)
                    nc.tensor.transpose(
                        tp[:], ex[:, b * C:(b + 1) * C], ident[0:C2, 0:C2]
                    )
                    nc.vector.tensor_copy(
                        kv_soft[0:C, b * C2:(b + 1) * C2], tp[:]
                    )
                nc.sync.dma_start(kv_soft[64:64 + C, :], kv_soft[0:C, :])
                stgs = {}
                for b in range(N):
                    stgs[b] = stage_pool.tile(
                        [128, (NSL // 128) * C2], F32, tag=f"stg{b}",
                        name=f"stg{b}",
                    )
                # Row-tiled phase 2: tile j contracts on PE rows 0:64
                # (kct rows 0:64, kv rows 0:64), tile j+8 on rows 64:128 —
                # the two matmuls run concurrently on separate subarrays.
                HNSL = NSL // 2
                for b in range(N):
                    for j in range(8):
                        col = slice(b * HNSL + j * 128, b * HNSL + (j + 1) * 128)
                        o_a = ps.tile(
                            [128, C2], F32, tag="o", name=f"oa{b}_{j}", bufs=4
                        )
                        nc.tensor.matmul(
                            o_a[:],
                            lhsT=kct_sb[0:C, col],
                            rhs=kv_soft[0:C, b * C2:(b + 1) * C2],
                            start=True,
                            stop=True,
                            tile_position=(0, 0),
                        )
                        o_b = ps.tile(
                            [128, C2], F32, tag="o", name=f"ob{b}_{j}", bufs=4
                        )
                        nc.tensor.matmul(
                            o_b[:],
                            lhsT=kct_sb[64:64 + C, col],
                            rhs=kv_soft[64:64 + C, b * C2:(b + 1) * C2],
                            start=True,
                            stop=True,
                            tile_position=(64, 0),
                        )
                        nc.vector.tensor_add(
                            stgs[b][:, j * C2:(j + 1) * C2],
                            o_a[:],
                            vc_sb[:, b * 1024 + j * C2: b * 1024 + (j + 1) * C2],
                        )
                        nc.vector.tensor_add(
                            stgs[b][:, (j + 8) * C2:(j + 9) * C2],
                            o_b[:],
                            vc_sb[:, b * 1024 + (j + 8) * C2: b * 1024 + (j + 9) * C2],
                        )
                    # split the store so the second half overlaps the
                    # remaining adds (trims the last batch's tail)
                    oap = out[b].rearrange("(p j) c -> p (j c)", p=128)
                    nc.sync.dma_start(oap[:, 0:8 * C2], stgs[b][:, 0:8 * C2])
                    nc.sync.dma_start(
                        oap[:, 8 * C2:16 * C2], stgs[b][:, 8 * C2:16 * C2]
                    )

            # ---- phase 1: partial kvT[d, c] per batch, col-tiled 2x ----
            # Even token-tiles accumulate on PE column group 0 (psum rows
            # 0:64), odd tiles on column group 2 (psum rows 64:128); the two
            # halves' LDWEIGHTS/MATMUL overlap on independent subarrays.
            for b in range(N):
                kv_ps = ps.tile([128, C], F32, tag="kv", name=f"kv{b}")
                for h in range(2):
                    k_sb = big.tile([128, A_TILES, C], F16, tag="k")
                    v_sb = big.tile([128, A_TILES, C2], F16, tag="v")
                    nc.sync.dma_start(
                        k_sb[:], km[b, h].rearrange("p (a c) -> p a c", a=A_TILES)
                    )
                    nc.sync.dma_start(
                        v_sb[:], vm[b, h].rearrange("p (a c) -> p a c", a=A_TILES)
                    )
                    if h == 1:
                        # phase-2 inputs for batch b: issued on the scalar
                        # (ACT) DMA FIFO so they never delay the phase-1
                        # chunk stream on the sync FIFO.
                        nc.scalar.dma_start(
                            kct_sb[:, b * (NSL // 2):(b + 1) * (NSL // 2)],
                            kct[b],
                        )
                        nc.scalar.dma_start(
                            vc_sb[:, b * 1024:(b + 1) * 1024],
                            vc[b].rearrange("(p j) c -> p (j c)", p=128),
                        )
                    for a in range(A_TILES):
                        half = a % 2
                        nc.tensor.matmul(
                            kv_ps[64 * half:64 * half + C2, :],
                            lhsT=v_sb[:, a, :],
                            rhs=k_sb[:, a, :],
                            start=(h == 0 and a < 2),
                            stop=(h == 1 and a >= A_TILES - 2),
                            tile_position=(0, 64 * half),
                        )
                # partial kvT = even-half + odd-half (DVE can read only one
                # PSUM operand per instruction, so copy then add)
                nc.vector.tensor_copy(kvt_sb[:, b * C:(b + 1) * C], kv_ps[0:C2, :])
                nc.vector.tensor_add(
                    kvt_sb[:, b * C:(b + 1) * C],
                    kvt_sb[:, b * C:(b + 1) * C],
                    kv_ps[64:64 + C2, :],
                )
                # after every second batch, AllGather the 2-batch group
                # (cheaper than AllReduce on the CC core; the 8 partials are
                # tree-reduced locally on DVE). Two groups instead of four
                # per-batch collectives: the CC core serializes ops at
                # 7-19us each, so fewer+fatter wins.
                if b % 2 == 1:
                    g = b // 2
                    ar_in = dram.tile(
                        [C2, 2 * C], F32, tag=f"ar_in{g}", name=f"ar_in{g}"
                    )
                    ar_out = dram.tile(
                        [N_CORES, C2, 2 * C], F32, addr_space="Shared",
                        tag=f"ar_out{g}", name=f"ar_out{g}",
                    )
                    ar_outs[g] = ar_out
                    nc.scalar.dma_start(
                        ar_in[:], kvt_sb[:, (b - 1) * C:(b + 1) * C]
                    )
                    nc.gpsimd.collective_compute(
                        "AllGather",
                        mybir.AluOpType.bypass,
                        replica_groups=[list(range(N_CORES))],
                        ins=[ar_in.opt()],
                        outs=[ar_out.opt()],
                    )
            emit_tails()

    nc.compile()
    return nc


def _get_program():
    if "nc" not in _CACHE:
        _CACHE["nc"] = _build_program()
    return _CACHE["nc"]


def kernel(key_mem, val_mem, key_cur, val_cur, alpha):
    key_mem = np.asarray(key_mem, dtype=np.float32)
    val_mem = np.asarray(val_mem, dtype=np.float32)
    key_cur = np.asarray(key_cur, dtype=np.float32)
    val_cur = np.asarray(val_cur, dtype=np.float32)
    alpha_f = float(np.asarray(alpha).reshape(-1)[0])

    nc = _get_program()

    # key_cur^T with alpha folded in; token axis permuted so that SBUF
    # column j*128+p holds token p*16+j (phase-2 store contiguity).
    kc_scaled = (alpha_f * key_cur).astype(np.float32)
    in_maps = []
    for i in range(N_CORES):
        kct_i = kc_scaled[:, i * NSL:(i + 1) * NSL, :].transpose(0, 2, 1)
        kct_i = (
            kct_i.reshape(N, C, 128, NSL // 128)
            .transpose(0, 1, 3, 2)
            .reshape(N, C, NSL)
        )
        # pack for row-tiled phase 2: rows 0:64 = tiles j=0..7,
        # rows 64:128 = tiles j=8..15
        kct_i = (
            kct_i.reshape(N, C, 2, NSL // 2)
            .transpose(0, 2, 1, 3)
            .reshape(N, 128, NSL // 2)
        )
        # phase-1 inputs: [b, h, p, a, c] with token t = h*8192 + p*64 + a
        # is a pure reshape of the [b, t, c] slice (p-major), so the DMA
        # sees dense 8KB partition rows.
        km_i = (
            key_mem[:, i].reshape(N, 2, 128, A_TILES * C).astype(np.float16)
        )
        vm_i = (
            val_mem[:, i].reshape(N, 2, 128, A_TILES * C2).astype(np.float16)
        )
        in_maps.append(
            {
                "key_mem": np.ascontiguousarray(km_i),
                "val_mem": np.ascontiguousarray(vm_i),
                "key_curT": np.ascontiguousarray(kct_i.astype(np.float16)),
                "val_cur": np.ascontiguousarray(val_cur[:, i * NSL:(i + 1) * NSL, :]),
            }
        )

    res = bass_utils.run_bass_kernel_spmd(
        nc, in_maps, core_ids=list(range(N_CORES)), **_RUN_OPTS
    )
    _CACHE["last_result"] = res
    outs = [res.results[i]["out"] for i in range(N_CORES)]
    return np.concatenate(outs, axis=1).astype(np.float32)
